# revision 1
# baseline (speedup 1.0000x reference)
"""Crystalformer multihead attention (per-crystal dense blocks) on 8 TRN2 cores.

Problem structure: 64 crystals x 64 atoms; edges form a dense 64x64 block per
crystal, so the segment-softmax attention is per-crystal dense attention with a
per-edge additive logit bias (attn_weights) and a per-edge additive value term
(values, 512MB -- the dominant HBM stream).

Strategy (data-parallel over crystals, 8 crystals/core):
  * All softmax state is computed TRANSPOSED (pT[j, (h,i)]) so the AV matmul
    and the denominator come straight out of the TensorEngine with no p
    transposes. The upper 64 partitions hold pT shifted by one i-column so a
    [128,2] block-diagonal weight slice covers an i-pair.
  * The per-edge values stream is consumed directly by PE matmuls
    (lhsT = tiny [128,2] p-weight slices, rhs = raw DMA tiles), so neither
    DVE nor ACT ever touches the 512MB stream: the kernel is DMA-bound.
  * exp() is done without the max-subtraction: logits here are O(10) and
    exp() is mathematically shift-invariant after normalization, so fp32 is
    safe and exact to normal rounding.
"""

import sys
from contextlib import ExitStack

import numpy as np

sys.path.insert(0, "/opt/trn_rl_repo")

import concourse.bacc as bacc  # noqa: E402
import concourse.bass as bass  # noqa: E402
import concourse.tile as tile  # noqa: E402
from concourse import masks, mybir  # noqa: E402
from concourse.bass_utils import run_bass_kernel_spmd  # noqa: E402

G, NATOMS, H, DH = 64, 64, 8, 64
CORES = 8
GPC = G // CORES                 # crystals per core
ROWS = GPC * NATOMS              # atom rows per core
EROWS = GPC * NATOMS * NATOMS    # edge rows per core
F = H * DH                       # flattened (head, dh) = 512

VALS_BF16 = False                # perf lever: stream `values` as bf16

_NC_CACHE = {}


def build_nc(n_crystals=GPC, vals_bf16=VALS_BF16, vals_bufs=16):
    f32 = mybir.dt.float32
    vdt = mybir.dt.bfloat16 if vals_bf16 else f32
    rows = n_crystals * NATOMS
    erows = n_crystals * NATOMS * NATOMS

    # Bacc (not raw Bass): its compile() pass legalizes multi-wait
    # instructions (1-wait HW limit) by spilling onto ldweights/event-sems
    nc = bacc.Bacc()
    q_d = nc.declare_dram_parameter("q", [rows, F], f32, isOutput=False)
    k_d = nc.declare_dram_parameter("k", [rows, F], f32, isOutput=False)
    v_d = nc.declare_dram_parameter("v", [rows, F], f32, isOutput=False)
    aw_d = nc.declare_dram_parameter("aw", [rows, F], f32, isOutput=False)
    vals_d = nc.declare_dram_parameter("vals", [erows, F], vdt, isOutput=False)
    out_d = nc.declare_dram_parameter("out", [rows, F], f32, isOutput=True)

    PS = bass.MemorySpace.PSUM
    with tile.TileContext(nc) as tc, ExitStack() as ctx:
        const = ctx.enter_context(tc.tile_pool(name="const", bufs=1))
        io = ctx.enter_context(tc.tile_pool(name="io", bufs=2))
        work = ctx.enter_context(tc.tile_pool(name="work", bufs=2))
        valsp = ctx.enter_context(tc.tile_pool(name="valsp", bufs=vals_bufs))
        osb = ctx.enter_context(tc.tile_pool(name="osb", bufs=2))
        tps = ctx.enter_context(tc.tile_pool(name="tps", bufs=1, space=PS))
        sps = ctx.enter_context(tc.tile_pool(name="sps", bufs=2, space=PS))
        ups_pool = ctx.enter_context(tc.tile_pool(name="upsp", bufs=2, space=PS))
        dps = ctx.enter_context(tc.tile_pool(name="dps", bufs=1, space=PS))
        fps = ctx.enter_context(tc.tile_pool(name="fps", bufs=1, space=PS))

        ident = const.tile([128, 128], f32)
        masks.make_identity(nc, ident[:])
        ones = const.tile([64, 1], f32)
        nc.vector.memset(ones[:], 1.0)

        # PE matmuls can carry only ONE semaphore wait; absorb the identity's
        # producer tick into PE's vector clock with a throwaway transpose so
        # later matmuls only ever wait on their data DMA.
        sink0 = dps.tile([128, F], f32, tag="den")
        nc.tensor.transpose(sink0[0:1, 256:320], ident[0:64, 0:1],
                            ident[0:64, 0:64])

        for ci in range(n_crystals):
            r0 = ci * NATOMS
            m0 = ci * NATOMS * NATOMS

            q_t = io.tile([64, F], f32, tag="q")
            nc.gpsimd.dma_start(q_t[:], q_d[r0:r0 + 64, :])
            k_t = io.tile([64, F], f32, tag="k")
            nc.gpsimd.dma_start(k_t[:], k_d[r0:r0 + 64, :])
            v_t = io.tile([64, F], f32, tag="v")
            nc.gpsimd.dma_start(v_t[:], v_d[r0:r0 + 64, :])
            aw_t = io.tile([64, F], f32, tag="aw")
            nc.gpsimd.dma_start(aw_t[:], aw_d[r0:r0 + 64, :])

            # q, k per head: [i, d] -> [d, (h, i)] (contraction dim on partitions)
            tq = tps.tile([64, F], f32, tag="tq")
            tk = tps.tile([64, F], f32, tag="tk")
            for h in range(H):
                hs = slice(h * DH, (h + 1) * DH)
                nc.tensor.transpose(tq[:, hs], q_t[:, hs], ident[0:64, 0:64])
                nc.tensor.transpose(tk[:, hs], k_t[:, hs], ident[0:64, 0:64])
            qT = work.tile([64, F], f32, tag="qT")
            nc.vector.tensor_scalar_mul(qT[:], tq[:], 0.125)  # 1/sqrt(DH)
            kT = work.tile([64, F], f32, tag="kT")
            nc.vector.tensor_copy(kT[:], tk[:])

            # S[j, (h, i)] = K·Qscaled; upper partitions get the i+1 shift.
            # Bias B^T accumulates into the same PSUM via transpose-matmuls.
            # PSUM start/stop is bank-granular: exactly one start=True (first
            # matmul into the bank) and one stop=True (last); first write to
            # each element overwrites, later writes accumulate.
            S = sps.tile([128, F], f32, tag="S")
            awv = aw_t[:].rearrange("p (j h) -> p h j", h=H)
            for h in range(H):
                hs = slice(h * DH, (h + 1) * DH)
                nc.tensor.matmul(S[0:64, hs], lhsT=kT[:, hs], rhs=qT[:, hs],
                                 start=(h == 0), stop=False,
                                 skip_group_check=True)
                # upper half: i+1 shift; spill one column into the next head
                # (finite garbage, overwritten/never read) so col 63 is never
                # left as stale PSUM that exp() would see
                ncols = DH if h < H - 1 else DH - 1
                nc.tensor.matmul(S[64:128, h * DH:h * DH + ncols],
                                 lhsT=kT[:, hs],
                                 rhs=qT[:, h * DH + 1:h * DH + 1 + ncols],
                                 start=(h == 0), stop=False,
                                 skip_group_check=True)
            # bias: out[j, n] = sum_i B[i, j] * ident[i, n(+1)] = B[n(+1), j];
            # the column-sliced identity applies the i+1 shift for the upper half
            for h in range(H):
                hs = slice(h * DH, (h + 1) * DH)
                nc.tensor.matmul(S[0:64, hs], lhsT=awv[:, h, :],
                                 rhs=ident[0:64, 0:64],
                                 start=False, stop=(h == H - 1),
                                 skip_group_check=True)
                # ident[0:64, 1:65]: column 64 is all-zero in this partition
                # slice, so the upper col 63 gets bias 0 (finite, never read)
                nc.tensor.matmul(S[64:128, hs],
                                 lhsT=awv[:, h, :], rhs=ident[0:64, 1:65],
                                 start=False, stop=(h == H - 1),
                                 skip_group_check=True)
            # ACT absorber: observe DVE's latest tick (kT copy) so exp only
            # needs its PE wait (one-wait-per-instruction codegen limit)
            asink = work.tile([1, 8], f32, tag="asink")
            nc.scalar.copy(asink[0:1, 0:1], kT[0:1, 0:1])
            p2 = work.tile([128, F], f32, tag="p2")
            nc.scalar.activation(p2[:], S[:], mybir.ActivationFunctionType.Exp)
            # DVE absorber: observe ACT's exp tick before the pz copies
            dsink = work.tile([1, 8], f32, tag="dsink")
            nc.vector.tensor_copy(dsink[0:1, 0:1], p2[0:1, 0:1])

            # pz[p, (h, t, s)]: block-diagonal weights for the i-pair matmuls
            pz = work.tile([128, F], vdt, tag="pz")
            nc.vector.memset(pz[:], 0.0)
            pzv = pz[:].rearrange("p (h t s) -> p h t s", t=32, s=2)
            p2v = p2[:].rearrange("p (h t s) -> p h t s", t=32, s=2)
            nc.vector.tensor_copy(pzv[0:64, :, :, 0], p2v[0:64, :, :, 0])
            nc.vector.tensor_copy(pzv[64:128, :, :, 1], p2v[64:128, :, :, 0])

            # PE absorber: observe both pz copies' DVE ticks so each U matmul
            # waits only on its values-tile DMA (one-wait limit); pz[:, 0:2]
            # covers bytes from both strided copies
            den = dps.tile([128, F], f32, tag="den")
            nc.tensor.transpose(den[0:2, 256:320], pz[:, 0:2],
                                ident[:, 0:64])

            # U-part, transposed: vals tile is the STATIONARY, the [128,2]
            # p-weight slice is the moving operand, so the output lands at
            # PSUM base 0 as outT[d, (h, i)] in disjoint column pairs.
            outT = ups_pool.tile([64, F], f32, tag="outT")
            for t in range(32):
                vt = valsp.tile([128, F], vdt, tag="vt")
                nc.sync.dma_start(vt[:], vals_d[m0 + 128 * t:m0 + 128 * (t + 1), :])
                vtv = vt[:].rearrange("p (h d) -> p h d", d=DH)
                for h in range(H):
                    nc.tensor.matmul(outT[:, h * DH + 2 * t:h * DH + 2 * t + 2],
                                     lhsT=vtv[:, h, :], rhs=pzv[:, h, t, :],
                                     start=(t == 0 and h == 0), stop=False)

            vv = v_t[:].rearrange("p (h d) -> p h d", d=DH)
            for h in range(H):
                hs = slice(h * DH, (h + 1) * DH)
                nc.tensor.matmul(outT[:, hs], lhsT=vv[:, h, :],
                                 rhs=p2[0:64, hs], start=False, stop=(h == H - 1))
                nc.tensor.matmul(den[0:64, h:h + 1], lhsT=p2[0:64, hs],
                                 rhs=ones[:], start=True, stop=True)

            rden = work.tile([64, 8], f32, tag="rden")
            nc.vector.reciprocal(rden[:], den[0:64, 0:8])

            # back to [i, (h, d)]: copy to SBUF, transpose per head, normalize
            o_sbT = osb.tile([64, F], f32, tag="oT")
            nc.scalar.copy(o_sbT[:], outT[:])
            o_ps = fps.tile([64, F], f32, tag="ofin")
            for h in range(H):
                hs = slice(h * DH, (h + 1) * DH)
                nc.tensor.transpose(o_ps[:, hs], o_sbT[:, hs], ident[0:64, 0:64])
            # DVE absorber: observe the final transposes' PE ticks so the
            # normalize multiply only needs its same-engine wait
            o_psv = o_ps[:].rearrange("p (h d) -> p h d", d=DH)
            dsink2 = work.tile([1, 8], f32, tag="dsink")
            nc.vector.tensor_copy(dsink2[0:1, 0:8], o_psv[0:1, :, 0])
            o_sb = osb.tile([64, F], f32, tag="o")
            nc.vector.tensor_tensor(
                o_sb[:].rearrange("p (h d) -> p h d", d=DH),
                o_ps[:].rearrange("p (h d) -> p h d", d=DH),
                rden[:].unsqueeze(2).broadcast_to([64, H, DH]),
                op=mybir.AluOpType.mult)
            nc.scalar.dma_start(out_d[r0:r0 + 64, :], o_sb[:])
    if not nc.is_finalized():
        nc.finalize()
    return nc


def _get_nc():
    key = (GPC, VALS_BF16)
    if key not in _NC_CACHE:
        _NC_CACHE[key] = build_nc()
    return _NC_CACHE[key]


def _edges_are_dense_blocks(e):
    base = np.arange(G, dtype=np.int64)[:, None, None] * NATOMS
    idx = np.arange(NATOMS, dtype=np.int64)
    e0 = np.broadcast_to(base + idx[None, :, None], (G, NATOMS, NATOMS)).reshape(-1)
    e1 = np.broadcast_to(base + idx[None, None, :], (G, NATOMS, NATOMS)).reshape(-1)
    return np.array_equal(e[0], e0) and np.array_equal(e[1], e1)


def _numpy_fallback(q, k, v, attn_weights, values, edges):
    # general (arbitrary-edges) segment-softmax path; slow but exact
    N = q.shape[0]
    e0, e1 = edges[0].astype(np.int64), edges[1].astype(np.int64)
    a = np.einsum("mhd,mhd->mh", q[e0] / np.sqrt(DH), k[e1]) + attn_weights
    m = np.full((N, H), -np.inf, np.float32)
    np.maximum.at(m, e0, a)
    p = np.exp(a - m[e0])
    den = np.zeros((N, H), np.float32)
    np.add.at(den, e0, p)
    w = p / den[e0]
    out = np.zeros_like(q)
    np.add.at(out, e0, w[:, :, None] * (v[e1] + values))
    return out


def make_in_maps(q, k, v, attn_weights, values):
    if VALS_BF16:
        import ml_dtypes
        vals_all = values.reshape(G * NATOMS * NATOMS, F).astype(ml_dtypes.bfloat16)
    else:
        vals_all = values.reshape(G * NATOMS * NATOMS, F)

    in_maps = []
    for c in range(CORES):
        ra = slice(c * ROWS, (c + 1) * ROWS)
        re = slice(c * EROWS, (c + 1) * EROWS)
        in_maps.append({
            "q": np.ascontiguousarray(q[ra]).reshape(ROWS, F),
            "k": np.ascontiguousarray(k[ra]).reshape(ROWS, F),
            "v": np.ascontiguousarray(v[ra]).reshape(ROWS, F),
            "aw": np.ascontiguousarray(attn_weights[re]).reshape(ROWS, F),
            "vals": np.ascontiguousarray(vals_all[re]),
        })
    return in_maps


def run_hw(q, k, v, attn_weights, values, **spmd_kwargs):
    in_maps = make_in_maps(q, k, v, attn_weights, values)
    br = run_bass_kernel_spmd(_get_nc(), in_maps, list(range(CORES)),
                              **spmd_kwargs)
    out = np.concatenate(
        [r["out"].reshape(ROWS, H, DH) for r in br.results], axis=0)
    return out.astype(np.float32), br


def kernel(q, k, v, attn_weights, values, edges):
    q = np.asarray(q, dtype=np.float32)
    k = np.asarray(k, dtype=np.float32)
    v = np.asarray(v, dtype=np.float32)
    attn_weights = np.asarray(attn_weights, dtype=np.float32)
    values = np.asarray(values, dtype=np.float32)
    e = np.asarray(edges)
    if not _edges_are_dense_blocks(e):
        return _numpy_fallback(q, k, v, attn_weights, values, e)
    return run_hw(q, k, v, attn_weights, values)[0]



# revision 5
# speedup vs baseline: 2.9802x; 2.9802x over previous
"""Crystalformer multihead attention (per-crystal dense blocks) on 8 TRN2 cores.

Problem structure: 64 crystals x 64 atoms; edges form a dense 64x64 block per
crystal, so the segment-softmax attention is per-crystal dense attention with a
per-edge additive logit bias (attn_weights) and a per-edge additive value term
(values, 512MB -- the dominant HBM stream).

Strategy (data-parallel over crystals, 8 crystals/core):
  * The per-edge U-term out_u[i,:] = sum_j p[i,j] * values[i,j,:] is computed
    as  (DVE: pw = p (x) vals broadcast-multiply)  followed by  (PE: ones
    block-diagonal stationary x pw as the MOVING operand, ap=512) -- so the
    512MB stream runs through PE at 1 col/cycle instead of the
    stationary-load-bound tiny-matmul scheme.
  * values are streamed bf16 (host-converted) in a host-pretransposed layout
    [g, T, (s,j), (t2,d,h)]: heads innermost so the DVE broadcast-weight AP
    has innermost stride 1 -> 2x_1P DVE mode (all operands 2-byte packed).
  * p2b holds exp(S) transposed [j, (i,h)] in bf16; upper 64 partitions are
    the i+1-shifted copy so a 128-partition edge tile (2 atoms x 64 j) reads
    its two weight columns from one AP.
  * The i-pair results are routed to output rows via a column-windowed slice
    of a constant block-diagonal ones matrix (PSUM col_grp only allows output
    base partition 0/32/64/96); 32 matmuls accumulate zeros elsewhere into
    one [64, 512] group.
  * exp() without max-subtraction: logits are O(10), exp is shift-invariant
    after normalization, fp32 PSUM + bf16 p is safe at the 2e-2 tolerance.
"""

import sys
from contextlib import ExitStack

import numpy as np

sys.path.insert(0, "/opt/trn_rl_repo")

import concourse.bacc as bacc  # noqa: E402
import concourse.bass as bass  # noqa: E402
import concourse.tile as tile  # noqa: E402
from concourse import masks, mybir  # noqa: E402
from concourse.bass_utils import run_bass_kernel_spmd  # noqa: E402

G, NATOMS, H, DH = 64, 64, 8, 64
CORES = 8
GPC = G // CORES                 # crystals per core
ROWS = GPC * NATOMS              # atom rows per core
EROWS = GPC * NATOMS * NATOMS    # edge rows per core
F = H * DH                       # flattened feature width = 512
QT = 8                           # quad-tiles per crystal (8 atoms x 64 j each)
QCOLS = 4 * F                    # 2048 cols per quad tile (t2, d, h)

_NC_CACHE = {}


def build_nc(n_crystals=GPC):
    f32 = mybir.dt.float32
    bf16 = mybir.dt.bfloat16

    nc = bacc.Bacc()
    q_d = nc.declare_dram_parameter("q", [n_crystals * 64, F], f32, isOutput=False)
    k_d = nc.declare_dram_parameter("k", [n_crystals * 64, F], f32, isOutput=False)
    aw_d = nc.declare_dram_parameter("aw", [n_crystals * 64, F], f32, isOutput=False)
    v_d = nc.declare_dram_parameter("v", [n_crystals * 64, F], bf16, isOutput=False)
    vals_d = nc.declare_dram_parameter(
        "vals", [n_crystals * QT * 128, QCOLS], bf16, isOutput=False)
    out_d = nc.declare_dram_parameter("out", [n_crystals * 64, F], bf16,
                                      isOutput=True)

    PS = bass.MemorySpace.PSUM
    with tile.TileContext(nc) as tc, ExitStack() as ctx:
        const = ctx.enter_context(tc.tile_pool(name="const", bufs=1))
        io = ctx.enter_context(tc.tile_pool(name="io", bufs=2))
        work = ctx.enter_context(tc.tile_pool(name="work", bufs=2))
        valsp = ctx.enter_context(tc.tile_pool(name="valsp", bufs=6))
        pwp = ctx.enter_context(tc.tile_pool(name="pwp", bufs=4))
        osb = ctx.enter_context(tc.tile_pool(name="osb", bufs=2))
        # PSUM: tqk holds 2x [64,F] tiles (transpose outputs must start at
        # partition 0, so tq/tk each get their own bank) -> 4 banks; S and O
        # [128,F] tiles -> 2 banks each. Total exactly 8.
        tqk = ctx.enter_context(tc.tile_pool(name="tqk", bufs=2, space=PS))
        ps2 = ctx.enter_context(tc.tile_pool(name="ps2", bufs=2, space=PS))
        ps3 = ctx.enter_context(tc.tile_pool(name="ps3", bufs=2, space=PS))

        ident = const.tile([128, 128], f32)
        masks.make_identity(nc, ident[:])
        # routing matrix for the U-part: col 64 = ones on partitions 0:64,
        # col 65 = ones on partitions 64:128. A [:, 64-i0 : 128-i0] window
        # puts the i-pair (i0, i0+1) result into output rows i0, i0+1.
        route = const.tile([128, 128], bf16)
        nc.vector.memset(route[:], 0.0)
        nc.vector.memset(route[0:64, 64:65], 1.0)
        nc.vector.memset(route[64:128, 65:66], 1.0)
        ones64 = const.tile([64, 1], bf16)
        nc.vector.memset(ones64[:], 1.0)

        for ci in range(n_crystals):
            r0 = ci * 64

            q_t = io.tile([64, F], f32, tag="q")
            nc.gpsimd.dma_start(q_t[:], q_d[r0:r0 + 64, :])
            k_t = io.tile([64, F], f32, tag="k")
            nc.gpsimd.dma_start(k_t[:], k_d[r0:r0 + 64, :])
            aw_t = io.tile([64, F], f32, tag="aw")
            nc.gpsimd.dma_start(aw_t[:], aw_d[r0:r0 + 64, :])
            v_t = io.tile([64, F], bf16, tag="v")
            nc.gpsimd.dma_start(v_t[:], v_d[r0:r0 + 64, :])

            # q, k per head: [i, d] -> [d, (h, i)]
            tq = tqk.tile([64, F], f32, tag="tq")
            tk = tqk.tile([64, F], f32, tag="tk")
            for h in range(H):
                hs = slice(h * DH, (h + 1) * DH)
                nc.tensor.transpose(tq[:, hs], q_t[:, hs], ident[0:64, 0:64])
                nc.tensor.transpose(tk[:, hs], k_t[:, hs], ident[0:64, 0:64])
            qT_sb = work.tile([64, F], f32, tag="qT")
            nc.scalar.activation(qT_sb[:], tq[:],
                                 mybir.ActivationFunctionType.Copy, scale=0.125)
            kT_sb = work.tile([64, F], f32, tag="kT")
            nc.scalar.copy(kT_sb[:], tk[:])

            # S[j, (h, i)] = K . Qscaled + bias^T, per-head groups in one bank
            S = ps2.tile([128, F], f32, tag="S")
            awv = aw_t[:].rearrange("p (j h) -> p h j", h=H)
            for h in range(H):
                hs = slice(h * DH, (h + 1) * DH)
                nc.tensor.matmul(S[0:64, hs], lhsT=kT_sb[:, hs],
                                 rhs=qT_sb[:, hs], start=True, stop=False,
                                 skip_group_check=True)
                nc.tensor.matmul(S[0:64, hs], lhsT=awv[:, h, :],
                                 rhs=ident[0:64, 0:64], start=False, stop=True,
                                 skip_group_check=True)

            # p2b[j, (i, h)] = exp(S) in bf16 ((i,h) order via strided ACT
            # write); upper half = i+1 shift = +8 cols
            p2b = work.tile([128, F], bf16, tag="p2b")
            nc.scalar.activation(
                p2b[0:64, :].rearrange("p (i h) -> p h i", h=H),
                S[0:64, :].rearrange("p (h i) -> p h i", i=64),
                mybir.ActivationFunctionType.Exp)
            nc.vector.tensor_copy(p2b[64:128, 0:F - 8], p2b[0:64, 8:F])

            # denominators: den[i, h] = sum_j p2b[j, (i,h)] -> S rows 64:128
            p2bv = p2b[0:64, :].rearrange("p (i h) -> p h i", h=H)
            for h in range(H):
                nc.tensor.matmul(S[64:128, h:h + 1], lhsT=p2bv[:, h, :],
                                 rhs=ones64[:], start=True, stop=True,
                                 skip_group_check=True)
            rden = work.tile([64, 8], f32, tag="rden")
            nc.vector.reciprocal(rden[:], S[64:128, 0:8])

            # U-part: pw = p (x) vals on DVE, then ones-routed reduction on
            # PE with pw as the 512-col MOVING operand. One accumulation
            # group on O: 32 U matmuls (each routes an i-pair to its rows,
            # zeros elsewhere) then 8 AV matmuls (stride-8 col APs).
            O = ps3.tile([64, F], f32, tag="O")
            p2q = p2b[:].rearrange("p (q t2 s h) -> p q t2 s h", q=QT, t2=4, s=2)
            for T in range(QT):
                vt = valsp.tile([128, QCOLS], bf16, tag="vt")
                m0 = (ci * QT + T) * 128
                nc.sync.dma_start(vt[:], vals_d[m0:m0 + 128, :])
                pw = pwp.tile([128, QCOLS], bf16, tag="pw")
                w_ap = p2q[:, T, :, 0, :].unsqueeze(2).broadcast_to(
                    [128, 4, DH, H])
                nc.vector.tensor_tensor(
                    pw[:].rearrange("p (t2 d h) -> p t2 d h", t2=4, h=H),
                    vt[:].rearrange("p (t2 d h) -> p t2 d h", t2=4, h=H),
                    w_ap, op=mybir.AluOpType.mult)
                for t2 in range(4):
                    i0 = 8 * T + 2 * t2
                    nc.tensor.matmul(
                        O[:], lhsT=route[:, 64 - i0:128 - i0],
                        rhs=pw[:, t2 * F:(t2 + 1) * F],
                        start=(T == 0 and t2 == 0), stop=False,
                        skip_group_check=True)

            # AV-part: O[i, d*8+h] += sum_j p[j,(i,h)] v[j,(d,h)]
            ov = O[:].rearrange("p (d h) -> p h d", h=H)
            vv = v_t[:].rearrange("p (d h) -> p h d", h=H)
            for h in range(H):
                nc.tensor.matmul(ov[:, h, :], lhsT=p2bv[:, h, :],
                                 rhs=vv[:, h, :], start=False, stop=(h == H - 1),
                                 skip_group_check=True)

            # normalize by 1/den, store bf16 (d,h)
            o_sb = osb.tile([64, F], bf16, tag="o")
            nc.vector.tensor_tensor(
                o_sb[:].rearrange("p (d h) -> p d h", h=H),
                O[:].rearrange("p (d h) -> p d h", h=H),
                rden[:].unsqueeze(1).broadcast_to([64, DH, H]),
                op=mybir.AluOpType.mult)
            nc.gpsimd.dma_start(out_d[r0:r0 + 64, :], o_sb[:])
    if not nc.is_finalized():
        nc.finalize()
    return nc


def _get_nc():
    key = GPC
    if key not in _NC_CACHE:
        _NC_CACHE[key] = build_nc()
    return _NC_CACHE[key]


def _edges_are_dense_blocks(e):
    base = np.arange(G, dtype=np.int64)[:, None, None] * NATOMS
    idx = np.arange(NATOMS, dtype=np.int64)
    e0 = np.broadcast_to(base + idx[None, :, None], (G, NATOMS, NATOMS)).reshape(-1)
    e1 = np.broadcast_to(base + idx[None, None, :], (G, NATOMS, NATOMS)).reshape(-1)
    return np.array_equal(e[0], e0) and np.array_equal(e[1], e1)


def _numpy_fallback(q, k, v, attn_weights, values, edges):
    # general (arbitrary-edges) segment-softmax path; slow but exact
    N = q.shape[0]
    e0, e1 = edges[0].astype(np.int64), edges[1].astype(np.int64)
    a = np.einsum("mhd,mhd->mh", q[e0] / np.sqrt(DH), k[e1]) + attn_weights
    m = np.full((N, H), -np.inf, np.float32)
    np.maximum.at(m, e0, a)
    p = np.exp(a - m[e0])
    den = np.zeros((N, H), np.float32)
    np.add.at(den, e0, p)
    w = p / den[e0]
    out = np.zeros_like(q)
    np.add.at(out, e0, w[:, :, None] * (v[e1] + values))
    return out


def make_in_maps(q, k, v, attn_weights, values):
    import ml_dtypes
    bf = ml_dtypes.bfloat16
    N = G * NATOMS

    # values: (g, i, j, h, d) -> [g, T, (s, j), (t2, d, h)], bf16
    va = values.reshape(G, QT, 4, 2, NATOMS, H, DH)   # g, T, t2, s, j, h, d
    va = va.transpose(0, 1, 3, 4, 2, 6, 5)            # g, T, s, j, t2, d, h
    va = va.astype(bf).reshape(G, QT * 128, QCOLS)

    # v: (n, h, d) -> (n, (d, h)) bf16
    v2 = np.ascontiguousarray(
        v.reshape(N, H, DH).transpose(0, 2, 1)).astype(bf).reshape(N, F)

    in_maps = []
    for c in range(CORES):
        ra = slice(c * ROWS, (c + 1) * ROWS)
        rg = slice(c * GPC, (c + 1) * GPC)
        re = slice(c * EROWS, (c + 1) * EROWS)
        in_maps.append({
            "q": np.ascontiguousarray(q[ra]).reshape(ROWS, F),
            "k": np.ascontiguousarray(k[ra]).reshape(ROWS, F),
            "aw": np.ascontiguousarray(attn_weights[re]).reshape(ROWS, F),
            "v": np.ascontiguousarray(v2[ra]),
            "vals": np.ascontiguousarray(va[rg]).reshape(GPC * QT * 128, QCOLS),
        })
    return in_maps


def run_hw(q, k, v, attn_weights, values, **spmd_kwargs):
    in_maps = make_in_maps(q, k, v, attn_weights, values)
    br = run_bass_kernel_spmd(_get_nc(), in_maps, list(range(CORES)),
                              **spmd_kwargs)
    # out rows are (d, h)-ordered bf16; convert + reorder to (h, d) fp32
    out = np.concatenate(
        [np.asarray(r["out"], dtype=np.float32) for r in br.results], axis=0)
    out = out.reshape(G * NATOMS, DH, H).transpose(0, 2, 1)
    return np.ascontiguousarray(out), br


def kernel(q, k, v, attn_weights, values, edges):
    q = np.asarray(q, dtype=np.float32)
    k = np.asarray(k, dtype=np.float32)
    v = np.asarray(v, dtype=np.float32)
    attn_weights = np.asarray(attn_weights, dtype=np.float32)
    values = np.asarray(values, dtype=np.float32)
    e = np.asarray(edges)
    if not _edges_are_dense_blocks(e):
        return _numpy_fallback(q, k, v, attn_weights, values, e)
    return run_hw(q, k, v, attn_weights, values)[0]


# revision 9
# speedup vs baseline: 3.7435x; 1.2561x over previous
"""Crystalformer multihead attention (per-crystal dense blocks) on 8 TRN2 cores.

Problem structure: 64 crystals x 64 atoms; edges form a dense 64x64 block per
crystal, so the segment-softmax attention is per-crystal dense attention with a
per-edge additive logit bias (attn_weights) and a per-edge additive value term
(values, 512MB -- the dominant HBM stream).

Strategy (data-parallel over crystals, 8 crystals/core):
  * The per-edge U-term out_u[i,:] = sum_j p[i,j] * values[i,j,:] is computed
    as  (DVE: pw = p (x) vals broadcast-multiply)  followed by  (PE: ones
    block-diagonal stationary x pw as the MOVING operand, ap=512) -- so the
    512MB stream runs through PE at 1 col/cycle instead of the
    stationary-load-bound tiny-matmul scheme.
  * values are streamed bf16 (host-converted) in a host-pretransposed layout
    [g, T, (s,j), (t2,d,h)]: heads innermost so the DVE broadcast-weight AP
    has innermost stride 1 -> 2x_1P DVE mode (all operands 2-byte packed).
  * p2b holds exp(S) transposed [j, (i,h)] in bf16; upper 64 partitions are
    the i+1-shifted copy so a 128-partition edge tile (2 atoms x 64 j) reads
    its two weight columns from one AP.
  * The i-pair results are routed to output rows via a column-windowed slice
    of a constant block-diagonal ones matrix (PSUM col_grp only allows output
    base partition 0/32/64/96); 32 matmuls accumulate zeros elsewhere into
    one [64, 512] group.
  * exp() without max-subtraction: logits are O(10), exp is shift-invariant
    after normalization, fp32 PSUM + bf16 p is safe at the 2e-2 tolerance.
"""

import sys
from contextlib import ExitStack

import numpy as np

sys.path.insert(0, "/opt/trn_rl_repo")

import concourse.bacc as bacc  # noqa: E402
import concourse.bass as bass  # noqa: E402
import concourse.tile as tile  # noqa: E402
from concourse import masks, mybir  # noqa: E402
from concourse.bass_utils import run_bass_kernel_spmd  # noqa: E402

G, NATOMS, H, DH = 64, 64, 8, 64
CORES = 8
GPC = G // CORES                 # crystals per core
ROWS = GPC * NATOMS              # atom rows per core
EROWS = GPC * NATOMS * NATOMS    # edge rows per core
F = H * DH                       # flattened feature width = 512
QT = 8                           # quad-tiles per crystal (8 atoms x 64 j each)
QCOLS = 4 * F                    # 2048 cols per quad tile (t2, d, h)

_NC_CACHE = {}


def build_nc(n_crystals=GPC):
    f32 = mybir.dt.float32
    bf16 = mybir.dt.bfloat16

    nc = bacc.Bacc()
    # qT/kT rows are d (host-transposed), cols (h, i); awT rows j, cols (h, i)
    qT_d = nc.declare_dram_parameter("qT", [n_crystals * 64, F], f32, isOutput=False)
    kT_d = nc.declare_dram_parameter("kT", [n_crystals * 64, F], f32, isOutput=False)
    awT_d = nc.declare_dram_parameter("awT", [n_crystals * 64, F], f32, isOutput=False)
    v_d = nc.declare_dram_parameter("v", [n_crystals * 64, F], bf16, isOutput=False)
    vals_d = nc.declare_dram_parameter(
        "vals", [n_crystals * QT * 128, QCOLS], bf16, isOutput=False)
    out_d = nc.declare_dram_parameter("out", [n_crystals * 64, F], bf16,
                                      isOutput=True)

    PS = bass.MemorySpace.PSUM
    with tile.TileContext(nc) as tc, ExitStack() as ctx:
        const = ctx.enter_context(tc.tile_pool(name="const", bufs=1))
        io = ctx.enter_context(tc.tile_pool(name="io", bufs=2))
        work = ctx.enter_context(tc.tile_pool(name="work", bufs=2))
        valsp = ctx.enter_context(tc.tile_pool(name="valsp", bufs=6))
        pwp = ctx.enter_context(tc.tile_pool(name="pwp", bufs=4))
        osb = ctx.enter_context(tc.tile_pool(name="osb", bufs=2))
        # PSUM: S [128,F] and O [64,F], 3 bufs each = 6 banks
        ps2 = ctx.enter_context(tc.tile_pool(name="ps2", bufs=3, space=PS))
        ps3 = ctx.enter_context(tc.tile_pool(name="ps3", bufs=3, space=PS))

        # routing matrix for the U-part: col 64 = ones on partitions 0:64,
        # col 65 = ones on partitions 64:128. A [:, 64-i0 : 128-i0] window
        # puts the i-pair (i0, i0+1) result into output rows i0, i0+1.
        route = const.tile([128, 128], bf16)
        nc.vector.memset(route[:], 0.0)
        nc.vector.memset(route[0:64, 64:65], 1.0)
        nc.vector.memset(route[64:128, 65:66], 1.0)
        ones64 = const.tile([64, 1], bf16)
        nc.vector.memset(ones64[:], 1.0)

        for ci in range(n_crystals):
            r0 = ci * 64

            qT_t = io.tile([64, F], f32, tag="qT")
            nc.gpsimd.dma_start(qT_t[:], qT_d[r0:r0 + 64, :])
            kT_t = io.tile([64, F], f32, tag="kT")
            nc.gpsimd.dma_start(kT_t[:], kT_d[r0:r0 + 64, :])
            awT_t = io.tile([64, F], f32, tag="awT")
            nc.gpsimd.dma_start(awT_t[:], awT_d[r0:r0 + 64, :])
            v_t = io.tile([64, F], bf16, tag="v")
            nc.gpsimd.dma_start(v_t[:], v_d[r0:r0 + 64, :])

            # q scaling on ACT (q/k arrive host-transposed [d, (h,i)])
            qTs = work.tile([64, F], f32, tag="qTs")
            nc.scalar.activation(qTs[:], qT_t[:],
                                 mybir.ActivationFunctionType.Copy, scale=0.125)

            # S[j, (h, i)] = K . Qscaled per head
            S = ps2.tile([128, F], f32, tag="S")
            for h in range(H):
                hs = slice(h * DH, (h + 1) * DH)
                nc.tensor.matmul(S[0:64, hs], lhsT=kT_t[:, hs],
                                 rhs=qTs[:, hs], start=True, stop=True,
                                 skip_group_check=True)

            # bias add on DVE (awT host-transposed to [j, (h,i)])
            S2 = work.tile([64, F], f32, tag="S2")
            nc.vector.tensor_tensor(S2[:], S[0:64, :], awT_t[:],
                                    op=mybir.AluOpType.add)

            # p2b[j, (i, h)] = exp(S2) in bf16 ((i,h) order via strided ACT
            # write); upper half = i+1 shift = +8 cols
            p2b = work.tile([128, F], bf16, tag="p2b")
            nc.scalar.activation(
                p2b[0:64, :].rearrange("p (i h) -> p h i", h=H),
                S2[:].rearrange("p (h i) -> p h i", i=64),
                mybir.ActivationFunctionType.Exp)
            nc.vector.tensor_copy(p2b[64:128, 0:F - 8], p2b[0:64, 8:F])

            # denominators: den[i, h] = sum_j p2b[j, (i,h)] -> S rows 64:128
            p2bv = p2b[0:64, :].rearrange("p (i h) -> p h i", h=H)
            for h in range(H):
                nc.tensor.matmul(S[64:128, h:h + 1], lhsT=p2bv[:, h, :],
                                 rhs=ones64[:], start=True, stop=True,
                                 skip_group_check=True)
            rden = work.tile([64, 8], f32, tag="rden")
            nc.vector.reciprocal(rden[:], S[64:128, 0:8])

            # U-part: pw = p (x) vals on DVE, then ones-routed reduction on
            # PE with pw as the 512-col MOVING operand. One accumulation
            # group on O: 32 U matmuls (each routes an i-pair to its rows,
            # zeros elsewhere) then 8 AV matmuls (stride-8 col APs).
            O = ps3.tile([64, F], f32, tag="O")
            p2q = p2b[:].rearrange("p (q t2 s h) -> p q t2 s h", q=QT, t2=4, s=2)
            for T in range(QT):
                vt = valsp.tile([128, QCOLS], bf16, tag="vt")
                m0 = (ci * QT + T) * 128
                nc.sync.dma_start(vt[:], vals_d[m0:m0 + 128, :])
                pw = pwp.tile([128, QCOLS], bf16, tag="pw")
                w_ap = p2q[:, T, :, 0, :].unsqueeze(2).broadcast_to(
                    [128, 4, DH, H])
                nc.vector.tensor_tensor(
                    pw[:].rearrange("p (t2 d h) -> p t2 d h", t2=4, h=H),
                    vt[:].rearrange("p (t2 d h) -> p t2 d h", t2=4, h=H),
                    w_ap, op=mybir.AluOpType.mult)
                for t2 in range(4):
                    i0 = 8 * T + 2 * t2
                    nc.tensor.matmul(
                        O[:], lhsT=route[:, 64 - i0:128 - i0],
                        rhs=pw[:, t2 * F:(t2 + 1) * F],
                        start=(T == 0 and t2 == 0), stop=False,
                        skip_group_check=True)

            # AV-part: O[i, d*8+h] += sum_j p[j,(i,h)] v[j,(d,h)]
            ov = O[:].rearrange("p (d h) -> p h d", h=H)
            vv = v_t[:].rearrange("p (d h) -> p h d", h=H)
            for h in range(H):
                nc.tensor.matmul(ov[:, h, :], lhsT=p2bv[:, h, :],
                                 rhs=vv[:, h, :], start=False, stop=(h == H - 1),
                                 skip_group_check=True)

            # normalize by 1/den, store bf16 (d,h)
            o_sb = osb.tile([64, F], bf16, tag="o")
            nc.vector.tensor_tensor(
                o_sb[:].rearrange("p (d h) -> p d h", h=H),
                O[:].rearrange("p (d h) -> p d h", h=H),
                rden[:].unsqueeze(1).broadcast_to([64, DH, H]),
                op=mybir.AluOpType.mult)
            nc.gpsimd.dma_start(out_d[r0:r0 + 64, :], o_sb[:])
    if not nc.is_finalized():
        nc.finalize()
    return nc


def _get_nc():
    key = GPC
    if key not in _NC_CACHE:
        _NC_CACHE[key] = build_nc()
    return _NC_CACHE[key]


def _edges_are_dense_blocks(e):
    base = np.arange(G, dtype=np.int64)[:, None, None] * NATOMS
    idx = np.arange(NATOMS, dtype=np.int64)
    e0 = np.broadcast_to(base + idx[None, :, None], (G, NATOMS, NATOMS)).reshape(-1)
    e1 = np.broadcast_to(base + idx[None, None, :], (G, NATOMS, NATOMS)).reshape(-1)
    return np.array_equal(e[0], e0) and np.array_equal(e[1], e1)


def _numpy_fallback(q, k, v, attn_weights, values, edges):
    # general (arbitrary-edges) segment-softmax path; slow but exact
    N = q.shape[0]
    e0, e1 = edges[0].astype(np.int64), edges[1].astype(np.int64)
    a = np.einsum("mhd,mhd->mh", q[e0] / np.sqrt(DH), k[e1]) + attn_weights
    m = np.full((N, H), -np.inf, np.float32)
    np.maximum.at(m, e0, a)
    p = np.exp(a - m[e0])
    den = np.zeros((N, H), np.float32)
    np.add.at(den, e0, p)
    w = p / den[e0]
    out = np.zeros_like(q)
    np.add.at(out, e0, w[:, :, None] * (v[e1] + values))
    return out


def make_in_maps(q, k, v, attn_weights, values):
    import ml_dtypes
    bf = ml_dtypes.bfloat16
    N = G * NATOMS

    # values: (g, i, j, h, d) -> [g, T, (s, j), (t2, d, h)], bf16
    va = values.reshape(G, QT, 4, 2, NATOMS, H, DH)   # g, T, t2, s, j, h, d
    va = va.transpose(0, 1, 3, 4, 2, 6, 5)            # g, T, s, j, t2, d, h
    va = va.astype(bf).reshape(G, QT * 128, QCOLS)

    # v: (n, h, d) -> (n, (d, h)) bf16
    v2 = np.ascontiguousarray(
        v.reshape(N, H, DH).transpose(0, 2, 1)).astype(bf).reshape(N, F)
    # qT/kT: (g, i, h, d) -> (g, d, (h, i)) fp32
    qT = np.ascontiguousarray(
        q.reshape(G, NATOMS, H, DH).transpose(0, 3, 2, 1)).reshape(N, F)
    kT = np.ascontiguousarray(
        k.reshape(G, NATOMS, H, DH).transpose(0, 3, 2, 1)).reshape(N, F)
    # awT: (g, i, j, h) -> (g, j, (h, i)) fp32
    awT = np.ascontiguousarray(
        attn_weights.reshape(G, NATOMS, NATOMS, H).transpose(0, 2, 3, 1)
    ).reshape(N, F)

    in_maps = []
    for c in range(CORES):
        ra = slice(c * ROWS, (c + 1) * ROWS)
        rg = slice(c * GPC, (c + 1) * GPC)
        in_maps.append({
            "qT": qT[ra],
            "kT": kT[ra],
            "awT": awT[ra],
            "v": np.ascontiguousarray(v2[ra]),
            "vals": np.ascontiguousarray(va[rg]).reshape(GPC * QT * 128, QCOLS),
        })
    return in_maps


def run_hw(q, k, v, attn_weights, values, **spmd_kwargs):
    in_maps = make_in_maps(q, k, v, attn_weights, values)
    br = run_bass_kernel_spmd(_get_nc(), in_maps, list(range(CORES)),
                              **spmd_kwargs)
    # out rows are (d, h)-ordered bf16; convert + reorder to (h, d) fp32
    out = np.concatenate(
        [np.asarray(r["out"], dtype=np.float32) for r in br.results], axis=0)
    out = out.reshape(G * NATOMS, DH, H).transpose(0, 2, 1)
    return np.ascontiguousarray(out), br


def kernel(q, k, v, attn_weights, values, edges):
    q = np.asarray(q, dtype=np.float32)
    k = np.asarray(k, dtype=np.float32)
    v = np.asarray(v, dtype=np.float32)
    attn_weights = np.asarray(attn_weights, dtype=np.float32)
    values = np.asarray(values, dtype=np.float32)
    e = np.asarray(edges)
    if not _edges_are_dense_blocks(e):
        return _numpy_fallback(q, k, v, attn_weights, values, e)
    return run_hw(q, k, v, attn_weights, values)[0]
